# revision 1
# baseline (speedup 1.0000x reference)
"""Trainium2 Bass kernel for the intra-batch point-cloud contrastive loss.

Math (matches the reference exactly):
  feats   = features_in.reshape(C, M).T    (row-major reinterpret), M = B*N
  labels  = labels_in.reshape(-1)
  sel     = bernoulli(key 42, min(750/(count+1),1)[labels])   (host, jax CPU)
  nv      = feats / ||feats||
  dp      = exp(nv @ nv.T / TEMP), diagonal zeroed
  pos_i   = sum_{j sel, same class} dp_ij ; neg over different class
  loss    = mean over selected i of -log(pos/(pos+neg))

Only selected points contribute (unselected rows give 0 loss, unselected
columns have zero weight), so the device works on the compacted point set
(~37% of M).  Columns are sharded over 8 cores.  To keep the program
SPMD-identical, each core receives the compacted feature matrix *rolled*
so its own columns sit first; the diagonal then lands at a fixed position
for every core and is zeroed with one shared (1-eye) mask.

Per core (L = M_pad/8 local columns, nT = M_pad/128 row chunks):
  mm1 (PE):  G_t = nvT[:, chunk t].T @ nvT[:, :L]     [128, L] fp32 psum
  exp (ACT): dp_t = exp(G_t / TEMP)                    -> SBUF
  mask(DVE): zero the diagonal sub-block (t < L/128)
  mm2 (PE):  S += W_t.T @ dp_t   (W = sel*onehot(label), [4, L] psum accum)
The host gathers the per-core S blocks and finishes the O(n_sel) epilogue.
"""

import numpy as np

TEMP = 0.07
NUM_CLASSES = 4
N_CORES = 8
P = 128

_NEFF_CACHE = {}


def _compute_sel(labels_flat):
    """Selection mask, bit-exact with the reference (jax threefry, key 42)."""
    import jax
    import jax.numpy as jnp

    cpu = jax.devices("cpu")[0]
    with jax.default_device(cpu):
        lab_j = jnp.asarray(labels_flat)
        counts = jnp.bincount(lab_j, length=NUM_CLASSES)
        keep_p = jnp.minimum(750.0 / (counts.astype(jnp.float32) + 1.0), 1.0)
        p = keep_p[lab_j]
        sel = jax.random.bernoulli(jax.random.key(42), p)
        return np.asarray(sel)


def _build_kernel(M_pad):
    import concourse.bass as bass
    import concourse.mybir as mybir
    import concourse.tile as tile

    L = M_pad // N_CORES          # local columns per core
    nT = M_pad // P               # 128-row chunks
    nL = L // P                   # chunks containing this core's diagonal
    f32 = mybir.dt.float32

    # consts layout (single tensor -> single DMA -> single semaphore):
    # cols [0, nT*4)                     W chunks (mm2 lhsT)
    # cols [nT*4, nT*4+P)                128x128 identity
    # cols [nT*4+P, nT*4+P+2L-P)         dwide (-1e9 shifted diagonal)
    CW = nT * NUM_CLASSES + P + (2 * L - P)
    o_eye = nT * NUM_CLASSES
    o_dw = o_eye + P

    nc = bass.Bass()
    packed_d = nc.dram_tensor("packed", [P, M_pad + CW], f32, kind="ExternalInput")
    s_d = nc.dram_tensor("s_out", [NUM_CLASSES, L], f32, kind="ExternalOutput")

    with tile.TileContext(nc) as tc:
        with (
            tc.tile_pool(name="singles", bufs=1) as singles,
            tc.tile_pool(name="dp", bufs=nT) as dp_pool,
            tc.tile_pool(name="ps", bufs=7, space="PSUM") as ps_pool,
            tc.tile_pool(name="acc", bufs=1, space="PSUM") as acc_pool,
        ):
            packed = singles.tile([P, M_pad + CW], f32)
            # ONE SWDGE (gpsimd) DMA -> one completion semaphore.  Several
            # DMAs (or an HWDGE multi-queue fan-out) would attach more inline
            # sync waits than this walrus build allows per instruction.
            nc.gpsimd.dma_start(out=packed[:], in_=packed_d[:])
            nvt = packed[0:64, 0:M_pad]
            w_sb = packed[:, M_pad + 0:M_pad + o_eye]
            eye_sb = packed[:, M_pad + o_eye:M_pad + o_dw]
            dwide_sb = packed[:, M_pad + o_dw:M_pad + CW]

            s_ps = acc_pool.tile([NUM_CLASSES, L], f32)
            rhs = nvt[:, 0:L]
            # dwide[p, c] = -1e9 iff c == p + (nL-1)*P; sliced so the -1e9
            # diagonal lands on this chunk's own columns [t*P, t*P+P).
            off0 = (nL - 1) * P
            for t in range(nT):
                ps = ps_pool.tile([P, L], f32)
                nc.tensor.matmul(
                    ps[:], nvt[:, t * P:(t + 1) * P], rhs,
                    start=True, stop=(t >= nL),
                )
                if t < nL:
                    # G += I.T @ D = D: pushes the diagonal to -1e9 so that
                    # exp() maps it to exactly 0.
                    nc.tensor.matmul(
                        ps[:], eye_sb, dwide_sb[:, off0 - t * P: off0 - t * P + L],
                        start=False, stop=True,
                    )
                dp = dp_pool.tile([P, L], f32)
                nc.scalar.activation(
                    dp[:], ps[:], mybir.ActivationFunctionType.Exp,
                    scale=float(1.0 / TEMP),
                )
                nc.tensor.matmul(
                    s_ps[:], w_sb[:, t * NUM_CLASSES:(t + 1) * NUM_CLASSES], dp[:],
                    start=(t == 0), stop=(t == nT - 1),
                )

            s_sb = singles.tile([NUM_CLASSES, L], f32)
            nc.scalar.copy(s_sb[:], s_ps[:])
            nc.gpsimd.dma_start(out=s_d[:], in_=s_sb[:])

    _split_multi_waits(nc)
    return nc


def _split_multi_waits(nc):
    """Walrus in this toolchain accepts only one inline sync-wait per
    instruction.  Tile's kernel-tail drain aggregates one wait per live
    semaphore, so hoist all but the last wait onto same-engine nops."""
    import concourse.mybir as mybir

    for fn in nc.m.functions:
        for blk in fn.blocks:
            insts = list(blk.instructions)
            out = []
            for inst in insts:
                si = inst.sync_info
                waits = list(si.on_wait) if si is not None and si.on_wait else []
                if len(waits) > 1:
                    for w in waits[:-1]:
                        out.append(mybir.InstNoOp(
                            name=nc.get_next_instruction_name(),
                            engine=inst.engine,
                            bass_nofuse=True,
                            sync_info=mybir.SyncInfo(on_wait=[w], on_update=[]),
                        ))
                    si.on_wait = waits[-1:]
                out.append(inst)
            if len(out) != len(insts):
                blk.instructions = out


def _get_kernel(M_pad):
    if M_pad not in _NEFF_CACHE:
        _NEFF_CACHE[M_pad] = _build_kernel(M_pad)
    return _NEFF_CACHE[M_pad]


def kernel(features_in, labels_in, _trace=False, _results=[None]):
    from concourse.bass_utils import run_bass_kernel_spmd

    features_in = np.asarray(features_in, dtype=np.float32)
    B, C, N = features_in.shape
    M = B * N
    labels = np.asarray(labels_in).reshape(-1).astype(np.int64)

    fT = features_in.reshape(C, M)                      # [C, M] reinterpret
    sel = _compute_sel(labels)
    idx = np.nonzero(sel)[0]
    n_sel = int(idx.size)
    n_div = max(n_sel, 1)

    norms = np.sqrt(np.sum(fT * fT, axis=0, dtype=np.float32)).astype(np.float32)
    nvT = (fT / norms).astype(np.float32)

    lab_sel = labels[idx]
    per_core = N_CORES * P
    M_pad = max(((n_sel + per_core - 1) // per_core) * per_core, per_core)
    L = M_pad // N_CORES
    nT = M_pad // P

    nvT_pad = np.zeros((C, M_pad), np.float32)
    nvT_pad[:, :n_sel] = nvT[:, idx]
    W = np.zeros((M_pad, NUM_CLASSES), np.float32)
    W[np.arange(n_sel), lab_sel] = 1.0

    nL = L // P
    eye = np.eye(P, dtype=np.float32)
    dwide = np.zeros((P, 2 * L - P), np.float32)
    dwide[np.arange(P), np.arange(P) + (nL - 1) * P] = -1e9

    in_maps = []
    for k in range(N_CORES):
        nv_k = np.ascontiguousarray(np.roll(nvT_pad, -L * k, axis=1))
        W_k = np.roll(W, -L * k, axis=0)
        # lhsT chunk t lives at columns [4t, 4t+4): w_arr[p, 4t+c] = W_k[128t+p, c]
        w_arr = W_k.reshape(nT, P, NUM_CLASSES).transpose(1, 0, 2).reshape(
            P, nT * NUM_CLASSES
        )
        consts = np.concatenate([w_arr, eye, dwide], axis=1)
        packed = np.zeros((P, M_pad + consts.shape[1]), np.float32)
        packed[:C, :M_pad] = nv_k
        packed[:, M_pad:] = consts
        in_maps.append({"packed": packed})

    nc = _get_kernel(M_pad)
    res = run_bass_kernel_spmd(nc, in_maps, core_ids=list(range(N_CORES)),
                               trace=_trace)
    _results[0] = res

    S = np.concatenate([res.results[k]["s_out"] for k in range(N_CORES)], axis=1)
    S = S[:, :n_sel]
    denom = np.sum(S, axis=0, dtype=np.float32).astype(np.float32)
    numer = S[lab_sel, np.arange(n_sel)]
    per = (-np.log(numer / denom)).astype(np.float32)
    loss = np.float32(per.sum(dtype=np.float32) / np.float32(n_div))
    return np.asarray(loss, dtype=np.float32)



# revision 8
# speedup vs baseline: 2.4690x; 2.4690x over previous
"""Trainium2 Bass kernel for the intra-batch point-cloud contrastive loss.

Math (matches the reference exactly):
  feats   = features_in.reshape(C, M).T    (row-major reinterpret), M = B*N
  labels  = labels_in.reshape(-1)
  sel     = bernoulli(key 42, min(750/(count+1),1)[labels])   (host, jax CPU)
  nv      = feats / ||feats||
  dp      = exp(nv @ nv.T / TEMP), diagonal zeroed
  pos_i   = sum_{j sel, same class} dp_ij ; neg over different class
  loss    = mean over selected i of -log(pos/(pos+neg))

Strategy (triangle/circulant, exp-minimal):
  Only selected points matter (~37% of M).  The selected points are
  class-sorted and each class padded to a 128-col boundary, so every
  128-block of columns is class-pure.  dp is symmetric, so each
  unordered block-pair is computed ONCE: row-block r covers col-blocks
  c = r+d (mod NB) for d in 0..NB/2.  Each core gets NB/8 row-blocks
  (circulant: core k takes global blocks {k, k+8, ...}; its input is
  rolled by k*128 so all slice offsets are SPMD-uniform).

  Per row-block window [128, W*128] (W = NB/2+1 blocks):
    PE : sim chunk = lhsT.T @ nv  (bf16, K=66: 64 feature rows + a
         "-1e9 * pad" row pair that masks pad rows/cols pre-exp)
         + eye@negeye matmul adding -1e9 on the d=0 diagonal
    ACT: dp = exp(sim/TEMP) -> bf16 SBUF  (the only exp; triangle
         halves the exp volume, which is the throughput wall)
    DVE: per-128-block row sums (3D-AP tensor_reduce) -> class-pure
         partials, class-mapped on the host
    PE : column sums of blocks d=1..W-2 via ones-vector matmul
         (the transpose-side contribution; d=0/W-1 excluded to avoid
         double counting)
  Host maps partials to classes and finishes the O(n_sel) epilogue.
"""

import numpy as np

TEMP = 0.07
NUM_CLASSES = 4
N_CORES = 8
P = 128

_NEFF_CACHE = {}


def _compute_sel(labels_flat):
    """Selection mask, bit-exact with the reference (jax threefry, key 42)."""
    import jax
    import jax.numpy as jnp

    cpu = jax.devices("cpu")[0]
    with jax.default_device(cpu):
        lab_j = jnp.asarray(labels_flat)
        counts = jnp.bincount(lab_j, length=NUM_CLASSES)
        keep_p = jnp.minimum(750.0 / (counts.astype(jnp.float32) + 1.0), 1.0)
        p = keep_p[lab_j]
        sel = jax.random.bernoulli(jax.random.key(42), p)
        return np.asarray(sel)


def _plan(NBp):
    """Chunk plan for one row-block window of W = NBp/2+1 blocks."""
    W = NBp // 2 + 1
    chunks = []
    m = 0
    while 8 * m < W:
        cw = min(8, W - 8 * m)            # blocks in this chunk
        lo = max(8 * m, 1)                # colsum block range [lo, hi)
        hi = min(8 * (m + 1), W - 1)
        chunks.append((m, cw, lo, hi))
        m += 1
    return W, chunks


def _build_kernel(NBp):
    import concourse.bass as bass
    import concourse.mybir as mybir
    import concourse.tile as tile

    Mp = NBp * P
    rpc = NBp // N_CORES                  # row-blocks per core
    W, chunks = _plan(NBp)
    cs_per_rb = (W - 2) * P               # colsum entries per row-block
    f32 = mybir.dt.float32
    bf16 = mybir.dt.bfloat16
    K = 66                                # 64 features + colmask + rowmask

    nc = bass.Bass()
    nv_d = nc.dram_tensor("nv", [K, Mp], bf16, kind="ExternalInput")
    lhs_d = nc.dram_tensor("lhs", [K, rpc * P], bf16, kind="ExternalInput")
    # consts: eye [0:128], negeye-wide [128:640] (-1e9 diag then zeros),
    # ones column [640:641]
    co_d = nc.dram_tensor("co", [P, 641], bf16, kind="ExternalInput")
    rs_d = nc.dram_tensor("rs_out", [P, rpc * W], f32, kind="ExternalOutput")
    cs_d = nc.dram_tensor("cs_out", [P, rpc * (W - 2)], f32,
                          kind="ExternalOutput")

    with tile.TileContext(nc) as tc:
        with (
            tc.tile_pool(name="singles", bufs=1) as singles,
            tc.tile_pool(name="dp", bufs=3) as dp_pool,
            tc.tile_pool(name="ps", bufs=3, space="PSUM") as ps_pool,
            tc.tile_pool(name="cs", bufs=1, space="PSUM") as cs_pool,
        ):
            co_sb = singles.tile([P, 641], bf16)
            lhs_sb = singles.tile([K, rpc * P], bf16)
            nv_sb = singles.tile([K, Mp], bf16)
            nc.gpsimd.dma_start(out=co_sb[:], in_=co_d[:])
            nc.gpsimd.dma_start(out=lhs_sb[:], in_=lhs_d[:])
            # split the big nv DMA so the first window's matmuls start early
            nch_dma = max(Mp // 1024, 1)
            for i in range(nch_dma):
                lo = i * (Mp // nch_dma)
                hi = (i + 1) * (Mp // nch_dma)
                nc.gpsimd.dma_start(out=nv_sb[:, lo:hi], in_=nv_d[:, lo:hi])

            eye = co_sb[:, 0:P]
            negw = co_sb[:, P:P + 512]
            ones_col = co_sb[:, 640:641]

            rs_sb = singles.tile([P, rpc * W], f32)
            cs_ps = cs_pool.tile([P, rpc * (W - 2)], f32)
            cs_sb = singles.tile([P, rpc * (W - 2)], f32)

            for b in range(rpc):
                lhs_b = lhs_sb[:, b * P:(b + 1) * P]
                dp = dp_pool.tile([P, W * P], bf16)
                for (m, cw, lo, hi) in chunks:
                    width = cw * P
                    src = ((b + m) * 1024) % Mp   # rolled col of chunk start
                    ps = ps_pool.tile([P, 1024], f32, name="ps")
                    # matmuls in <=512-col (one PSUM bank) pieces
                    off = 0
                    while off < width:
                        w512 = min(512, width - off)
                        first = m == 0 and off == 0
                        nc.tensor.matmul(
                            ps[:, off:off + w512],
                            lhs_b,
                            nv_sb[:, src + off:src + off + w512],
                            start=True,
                            stop=not first,
                        )
                        if first:
                            # adds -1e9 on the diagonal of window block 0
                            nc.tensor.matmul(
                                ps[:, 0:w512], eye, negw[:, 0:w512],
                                start=False, stop=True,
                            )
                        off += w512
                    nc.scalar.activation(
                        dp[:, m * 1024:m * 1024 + width], ps[:, 0:width],
                        mybir.ActivationFunctionType.Exp,
                        scale=float(1.0 / TEMP),
                    )
                    # per-block row sums (class-pure partials)
                    nc.vector.tensor_reduce(
                        rs_sb[:, b * W + 8 * m:b * W + 8 * m + cw],
                        dp[:, m * 1024:m * 1024 + width].rearrange(
                            "p (a b) -> p a b", b=P),
                        mybir.AxisListType.X,
                        mybir.AluOpType.add,
                    )
                    # column sums of blocks [lo, hi): dp block as stationary
                    # weights, ones as the 1-col moving operand -> out [128, 1]
                    for w in range(lo, hi):
                        doff = (w - 8 * m) * P + m * 1024
                        nc.tensor.matmul(
                            cs_ps[:, b * (W - 2) + w - 1:b * (W - 2) + w],
                            dp[:, doff:doff + P],
                            ones_col,
                            start=True, stop=True,
                        )
            nc.vector.tensor_scalar_add(cs_sb[:], cs_ps[:], 0.0)
            nc.gpsimd.dma_start(out=cs_d[:], in_=cs_sb[:])
            nc.gpsimd.dma_start(out=rs_d[:], in_=rs_sb[:])

    _split_multi_waits(nc)
    return nc


def _split_multi_waits(nc):
    """Walrus in this toolchain accepts only one inline sync-wait per
    instruction.  Tile's kernel-tail drain aggregates one wait per live
    semaphore, so hoist all but the last wait onto same-engine nops."""
    import concourse.mybir as mybir

    for fn in nc.m.functions:
        for blk in fn.blocks:
            insts = list(blk.instructions)
            out = []
            for inst in insts:
                si = inst.sync_info
                waits = list(si.on_wait) if si is not None and si.on_wait else []
                if len(waits) > 1:
                    for w in waits[:-1]:
                        out.append(mybir.InstNoOp(
                            name=nc.get_next_instruction_name(),
                            engine=inst.engine,
                            bass_nofuse=True,
                            sync_info=mybir.SyncInfo(on_wait=[w], on_update=[]),
                        ))
                    si.on_wait = waits[-1:]
                out.append(inst)
            if len(out) != len(insts):
                blk.instructions = out
    return nc


def _get_kernel(NBp):
    if NBp not in _NEFF_CACHE:
        _NEFF_CACHE[NBp] = _build_kernel(NBp)
    return _NEFF_CACHE[NBp]


def kernel(features_in, labels_in, _trace=False, _results=[None]):
    import ml_dtypes
    from concourse.bass_utils import run_bass_kernel_spmd

    bf16 = ml_dtypes.bfloat16
    features_in = np.asarray(features_in, dtype=np.float32)
    B, C, N = features_in.shape
    M = B * N
    labels = np.asarray(labels_in).reshape(-1).astype(np.int64)

    fT = features_in.reshape(C, M)                      # [C, M] reinterpret
    sel = _compute_sel(labels)
    idx = np.nonzero(sel)[0]
    lab_sel = labels[idx]
    n_sel = int(idx.size)
    n_div = max(n_sel, 1)

    # class-sorted, per-class 128-padded column layout
    order = np.argsort(lab_sel, kind="stable")
    idx_sorted = idx[order]
    lab_sorted = lab_sel[order]
    cnt = np.bincount(lab_sel, minlength=NUM_CLASSES)
    cls_blocks = np.maximum(np.ceil(cnt / P).astype(int), cnt > 0)
    NB = max(int(cls_blocks.sum()), 1)
    NBp = max(8 * int(np.ceil(NB / 8)), 8)
    Mp = NBp * P
    rpc = NBp // N_CORES
    W, chunks = _plan(NBp)
    cs_per_rb = (W - 2) * P

    norms = np.sqrt(np.sum(fT * fT, axis=0, dtype=np.float32))
    nvT = (fT / norms).astype(np.float32)

    col_of_point = np.zeros(n_sel, np.int64)
    block_class = np.full(NBp, -1, np.int64)
    nv = np.zeros((C, Mp), np.float32)
    padcol = np.ones(Mp, bool)
    b0 = 0
    pos_pt = 0
    for c in range(NUM_CLASSES):
        start = b0 * P
        n = int(cnt[c])
        sl = slice(pos_pt, pos_pt + n)
        col_of_point[sl] = start + np.arange(n)
        nv[:, start:start + n] = nvT[:, idx_sorted[sl]]
        padcol[start:start + n] = False
        block_class[b0:b0 + int(cls_blocks[c])] = c
        b0 += int(cls_blocks[c])
        pos_pt += n

    K = 66
    nv_ext = np.zeros((K, Mp), np.float32)
    nv_ext[:C] = nv
    nv_ext[C] = -1e9 * padcol                 # colmask row
    nv_ext[C + 1] = 1.0                       # pairs with lhs rowmask row

    eye = np.eye(P, dtype=np.float32)
    co = np.zeros((P, 641), np.float32)
    co[:, 0:P] = eye
    co[:, P:2 * P] = -1e9 * eye               # negeye (wide region zero-padded)
    co[:, 640] = 1.0                          # ones column

    in_maps = []
    for k in range(N_CORES):
        shift = -k * P
        nv_k = np.roll(nv_ext, shift, axis=1)
        lhs_k = np.zeros((K, rpc * P), np.float32)
        for b in range(rpc):
            cols = slice(b * 8 * P, b * 8 * P + P)
            lhs_k[:C, b * P:(b + 1) * P] = nv_k[:C, cols]
            lhs_k[C, b * P:(b + 1) * P] = 1.0
            lhs_k[C + 1, b * P:(b + 1) * P] = nv_k[C, cols]  # -1e9*padrow
        in_maps.append({
            "nv": nv_k.astype(bf16),
            "lhs": lhs_k.astype(bf16),
            "co": co.astype(bf16),
        })

    nc = _get_kernel(NBp)
    res = run_bass_kernel_spmd(nc, in_maps, core_ids=list(range(N_CORES)),
                               trace=_trace)
    _results[0] = res

    # host epilogue: map class-pure partials into S[class, col]
    S = np.zeros((NUM_CLASSES, Mp), np.float64)
    wblk = (np.arange(Mp) // P)               # col -> block
    for k in range(N_CORES):
        rs = np.asarray(res.results[k]["rs_out"], np.float64)
        cs = np.asarray(res.results[k]["cs_out"], np.float64)
        for b in range(rpc):
            r = (k + 8 * b) % NBp
            rows_glob = (np.arange(P) + 8 * b * P + k * P) % Mp
            for w in range(W):
                cls = block_class[(r + w) % NBp]
                if cls >= 0:
                    S[cls, rows_glob] += rs[:, b * W + w]
            cls_r = block_class[r]
            if cls_r >= 0:
                for w in range(1, W - 1):
                    c = (r + w) % NBp
                    S[cls_r, c * P:(c + 1) * P] += cs[:, b * (W - 2) + w - 1]

    pos = S[lab_sorted, col_of_point]
    denom = S[:, col_of_point].sum(axis=0)
    per = -np.log(pos / denom)
    loss = np.float32(per.sum() / np.float64(n_div))
    return np.asarray(loss, dtype=np.float32)
